# revision 6
# baseline (speedup 1.0000x reference)
"""Additive-attention pooling kernel for 8 TRN2 NeuronCores.

Problem (per full input):
    u = tanh(value @ W1^T + query @ W2^T + b)          # [B, S, H]
    scores = u @ w, masked to s < lens[b], softmax over s
    out = sum_s softmax(scores)[b, s] * value[b, s, :]  # [B, DV]

Sharding: data-parallel over the batch dim (4 batches per core); the small
parameters (W1, W2, b, w) are replicated.

Per-core pipeline (matmuls in bf16, f32 PSUM accumulation), software-
pipelined g-outer/b-inner so every phase overlaps the value-load DMA:
  1. SWDGE DMAs load value in 512KB chunks, casting f32->bf16 in the
     DMA datapath, into nat[p, t, v] = value[128t + p, v]; issue order is
     chunk-major/batch-minor so all four batches stream in lockstep.
  2. value^T tiles via DMA xbar transposes (SBUF->SBUF bf16 on the sync
     HWDGE queue): rides the 435GB/s SBUF fabric, which has headroom while
     HBM (358GB/s) is the load bottleneck, and keeps the PE free for real
     matmuls.  Output layout vt[vv, q=(t,vh), s].
  3. u-matmul per (batch, 1024-s chunk): W1T chunks stationary, vt moving;
     ScalarE tanh with per-partition bias (c = query@W2^T + b) -> uT bf16.
  4. After each chunk g completes across batches: scores matmuls (M=32,
     4 batches col-tiled concurrently), exp, xbar e-transpose (again DMA,
     straight to SBUF), DVE mask+replicate with accumulated e-sums.
  5. Pooling matmuls for chunk g's 8 s-tiles follow immediately (4 batches
     col-tiled, one PSUM bank per batch, accumulating over all 32 s-tiles).
  6. sum(e) -> reduce + one N=1 matmul per batch; reciprocal scale.
  Dummy warmup matmuls during the first loads release the PE HAM throttle
  before the real u-matmuls arrive.  All small parameters ship pre-packed
  in one [128, 1036] image so a single DMA replaces eight small ones.
"""

import numpy as np

import concourse.bass as bass
import concourse.bacc as bacc
import concourse.tile as tile
from concourse import mybir
from concourse.bass_utils import run_bass_kernel_spmd


B, S, DV, DQ, H = 32, 4096, 256, 256, 256
NCORES = 8
BL = B // NCORES  # batches per core

ST = S // 128     # 32 s-tiles of 128
NG = 4            # compute chunks per batch (1024 s each)
GT = ST // NG     # s-tiles per chunk (8)
PW = 1036         # packed params width: w1t 512 | w2t 512 | w 2 | b 2 | qT 8
F32 = mybir.dt.float32
BF16 = mybir.dt.bfloat16
I32 = mybir.dt.int32


def build_nc():
    nc = bacc.Bacc("TRN2", target_bir_lowering=False)

    value_ext = nc.declare_dram_parameter("value", [BL, S, DV], F32, isOutput=False)
    lens_ext = nc.declare_dram_parameter("lens", [BL], I32, isOutput=False)
    params_ext = nc.declare_dram_parameter(
        "params", [128, PW], F32, isOutput=False
    )
    out_ext = nc.declare_dram_parameter("out", [BL, DV], F32, isOutput=True)

    Tanh = mybir.ActivationFunctionType.Tanh
    Exp = mybir.ActivationFunctionType.Exp
    Alu = mybir.AluOpType

    with tile.TileContext(nc) as tc:
        with (
            tc.tile_pool(name="singles", bufs=1) as singles,
            tc.tile_pool(name="nat", bufs=BL) as nat_pool,
            tc.tile_pool(name="vt", bufs=8) as vt_pool,
            tc.tile_pool(name="et", bufs=2) as et_pool,
            tc.tile_pool(name="ut", bufs=2 * BL) as ut_pool,
        ):
            # s-index iota for the length mask: val[p, t] = 128t + p
            iota_s = singles.tile([128, ST], F32, tag="iota_s")
            nc.gpsimd.iota(
                iota_s, [[128, ST]], channel_multiplier=1,
                allow_small_or_imprecise_dtypes=True,
            )

            # ---- value loads: SWDGE cast-DMAs (f32->bf16), chunk-major/
            # batch-minor so the four batches' chunk g data all arrives early
            nat = []
            for b in range(BL):
                natb = nat_pool.tile([128, ST, DV], BF16, tag="nat")
                nat.append(natb)
            for ch in range(8):
                for b in range(BL):
                    src = value_ext[b, ch * 512:(ch + 1) * 512, :]
                    nc.gpsimd.dma_start(
                        out=nat[b][:, ch * 4:(ch + 1) * 4, :],
                        in_=src.rearrange("(t p) v -> p t v", p=128),
                    )

            params_sb = singles.tile([128, PW], F32, tag="params_sb")
            nc.sync.dma_start(out=params_sb, in_=params_ext[:, :])
            w1t_f = params_sb[:, 0:512].rearrange("p (c h) -> p c h", c=2)
            w2t_f = params_sb[:, 512:1024].rearrange("p (c h) -> p c h", c=2)
            w_f = params_sb[:, 1024:1026]
            b_sb = params_sb[:, 1026:1028]
            qT = params_sb[:, 1028:1036].rearrange("p (c b) -> p c b", c=2)

            lens_i = singles.tile([128, BL], I32, tag="lens_i")
            nc.sync.dma_start(
                out=lens_i,
                in_=bass.AP(tensor=lens_ext, offset=0, ap=[[0, 128], [1, BL]]),
            )
            lens_f = singles.tile([128, BL], F32, tag="lens_f")
            nc.vector.tensor_copy(lens_f, lens_i)

            w1t_bf = singles.tile([128, 2, H], BF16, tag="w1t_bf")
            nc.vector.tensor_copy(w1t_bf, w1t_f)

            zero32 = singles.tile([128, 32], BF16, tag="zero32")
            nc.vector.memset(zero32, 0.0)
            w_rep = singles.tile([128, 2, 32], BF16, tag="w_rep")
            for hh in range(2):
                nc.vector.tensor_scalar(
                    w_rep[:, hh, :], zero32, w_f[:, hh:hh + 1], None, Alu.add
                )

            # 1/32-filled stationary for the sum(e) matmul
            ones_rep = singles.tile([128, 32], BF16, tag="ones_rep")
            nc.vector.memset(ones_rep, 1.0 / 32.0)

            # c[b, h] = query[b] @ W2^T + b   ->  cT [128h, hh, b] f32
            # plus PE warmup: junk matmuls so the HAM clock-gate releases
            # (1.2 -> 2.4 GHz) while the first value chunks are still loading
            cT = singles.tile([128, 2, BL], F32, tag="cT")
            with tc.tile_pool(name="ct_ps", bufs=2, space="PSUM") as ct_pool:
                warm_ps = ct_pool.tile([128, 128], F32, tag="warm")
                for i in range(36):
                    nc.tensor.matmul(
                        warm_ps,
                        w2t_f[:, 0, 0:128],
                        w1t_f[:, 0, 0:128],
                        start=True,
                        stop=True,
                    )
                for hh in range(2):
                    ct_ps = ct_pool.tile([128, BL], F32, tag="ct")
                    for c in range(2):
                        nc.tensor.matmul(
                            ct_ps,
                            w2t_f[:, c, hh * 128:(hh + 1) * 128],
                            qT[:, c, :],
                            start=(c == 0),
                            stop=(c == 1),
                        )
                    nc.vector.tensor_scalar(
                        cT[:, hh, :], ct_ps, b_sb[:, hh:hh + 1], None, Alu.add
                    )

            # ---- main pipelined loop -----------------------------------
            ut = [None] * (2 * BL)
            e_sb = singles.tile([128, S], BF16, tag="e_sb")
            e_resh = singles.tile([128, ST, BL], BF16, tag="e_resh")
            e_rep = singles.tile([128, BL, ST, 32], BF16, tag="e_rep")
            psums = singles.tile([128, BL, 8], F32, tag="psums")
            psums_r = singles.tile([128, BL], F32, tag="psums_r")
            psums_bf = singles.tile([128, BL], BF16, tag="psums_bf")
            out_sb = singles.tile([128, DV], F32, tag="out_sb")
            sums_r = singles.tile([128, 1], F32, tag="sums_r")

            with (
                tc.tile_pool(name="up_ps", bufs=3, space="PSUM") as up_pool,
                tc.tile_pool(name="se_ps", bufs=1, space="PSUM") as se_pool,
                tc.tile_pool(name="po_ps", bufs=1, space="PSUM") as po_pool,
            ):
                # pooling accumulator: batch b col-tiled into partitions
                # [32b, 32b+32), bank b, cols [0, DV] (+ col DV = sum e)
                po_ps = po_pool.tile([128, BL, 512], F32, tag="po")

                def emit_e_chain(e8):
                    soff = e8 * 512
                    toff = e8 * 4
                    sc_ps = se_pool.tile([128, 512], F32, tag="se", name=f"sc{e8}")
                    for bb in range(BL):
                        for hh in range(2):
                            nc.tensor.matmul(
                                sc_ps[32 * bb:32 * bb + 32, :],
                                w_rep[:, hh, :],
                                ut[2 * bb + hh][:, soff:soff + 512],
                                start=(hh == 0),
                                stop=(hh == 1),
                                tile_position=(0, 32 * bb),
                            )
                    nc.scalar.activation(e_sb[:, soff:soff + 512], sc_ps, Exp)
                    # e-transpose via DMA xbar: et[x, q, k] = e_sb[k, soff+128q+x]
                    # (k = 32*bb + rep; column 32*bb holds batch bb's e values)
                    et = et_pool.tile([128, 4, 128], BF16, tag="et")
                    nc.sync.dma_start(
                        out=et,
                        in_=e_sb[:, soff:soff + 512],
                        transpose=True,
                    )
                    nc.vector.tensor_copy(
                        e_resh[:, toff:toff + 4, :],
                        et.rearrange("p q (bb x) -> p q bb x", x=32)[:, :, :, 0],
                    )
                    for bb in range(BL):
                        er = e_resh[:, toff:toff + 4, bb]
                        er_b = bass.AP(tensor=er.tensor, offset=er.offset,
                                       ap=[*er.ap, [0, 32]])
                        io = iota_s[:, toff:toff + 4]
                        io_b = bass.AP(tensor=io.tensor, offset=io.offset,
                                       ap=[*io.ap, [0, 32]])
                        nc.vector.scalar_tensor_tensor(
                            e_rep[:, bb, toff:toff + 4],
                            io_b,
                            lens_f[:, bb:bb + 1],
                            er_b,
                            Alu.is_lt,
                            Alu.mult,
                            accum_out=psums[:, bb, e8:e8 + 1],
                        )

                for g in range(NG):
                    t0 = g * GT
                    for b in range(BL):
                        # --- value^T tiles for (b, g): [vv, q=(t',vh), s]
                        vt = vt_pool.tile([128, 2 * GT, 128], BF16, tag="vt")
                        vt_q = vt.rearrange("p (t two) c -> p two t c", two=2)
                        nc.sync.dma_start(
                            out=vt,
                            in_=nat[b][:, t0:t0 + GT, :],
                            transpose=True,
                        )
                        # --- u-matmul + tanh for (b, g) ------------------
                        for hh in range(2):
                            if g == 0:
                                utb = ut_pool.tile([128, S], BF16, tag="ut")
                                ut[2 * b + hh] = utb
                            utb = ut[2 * b + hh]
                            for sc in range(2):
                                up = up_pool.tile([128, 512], F32, tag="up")
                                for vh in range(2):
                                    mv = vt_q[:, vh, sc * 4:(sc + 1) * 4, :]
                                    nc.tensor.matmul(
                                        up,
                                        w1t_bf[:, vh, hh * 128:(hh + 1) * 128],
                                        mv,
                                        start=(vh == 0),
                                        stop=(vh == 1),
                                    )
                                so = g * 1024 + sc * 512
                                nc.scalar.activation(
                                    utb[:, so:so + 512],
                                    up,
                                    Tanh,
                                    bias=cT[:, hh, b:b + 1],
                                    scale=1.0,
                                )

                    # --- scores / e for chunk g across all batches -------
                    emit_e_chain(2 * g)
                    emit_e_chain(2 * g + 1)

                    # --- pooling for chunk g's s-tiles -------------------
                    for k in range(GT):
                        t = t0 + k
                        for b in range(BL):
                            nc.tensor.matmul(
                                po_ps[32 * b:32 * b + 32, b, 0:DV],
                                e_rep[:, b, t, :],
                                nat[b][:, t, :],
                                start=(t == 0),
                                stop=(t == ST - 1),
                                tile_position=(0, 32 * b),
                            )

                # ---- finalize: sum(e), normalize, store ----------------
                nc.vector.tensor_reduce(
                    psums_r, psums, op=Alu.add, axis=mybir.AxisListType.X
                )
                nc.vector.tensor_copy(psums_bf, psums_r)
                for b in range(BL):
                    nc.tensor.matmul(
                        po_ps[32 * b:32 * b + 32, b, DV:DV + 1],
                        ones_rep,
                        psums_bf[:, b:b + 1],
                        start=True,
                        stop=True,
                        tile_position=(0, 32 * b),
                    )
                for b in range(BL):
                    rows = slice(32 * b, 32 * b + 32)
                    nc.vector.reciprocal(
                        sums_r[rows], po_ps[rows, b, DV:DV + 1]
                    )
                    nc.vector.tensor_scalar(
                        out_sb[rows], po_ps[rows, b, 0:DV], sums_r[rows],
                        None, Alu.mult,
                    )
                ob_rows = out_sb.rearrange("(a b) s -> a b s", b=32)[:, 0, :]
                nc.sync.dma_start(out=out_ext[:, :], in_=ob_rows)

    nc.compile()
    return nc


_NC_CACHE = None


def _get_nc():
    global _NC_CACHE
    if _NC_CACHE is None:
        _NC_CACHE = build_nc()
    return _NC_CACHE


def make_in_maps(value, query, lens, W1, W2, b, w):
    value = np.ascontiguousarray(np.asarray(value, dtype=np.float32))
    query = np.asarray(query, dtype=np.float32)
    lens = np.ascontiguousarray(np.asarray(lens, dtype=np.int32))
    w1t = np.asarray(W1, dtype=np.float32).T
    w2t = np.asarray(W2, dtype=np.float32).T
    bvec = np.asarray(b, dtype=np.float32).reshape(H)
    wvec = np.asarray(w, dtype=np.float32).reshape(H)

    def pack(core):
        sl = slice(core * BL, (core + 1) * BL)
        P = np.zeros((128, PW), np.float32)
        P[:, 0:512] = w1t.reshape(2, 128, H).transpose(1, 0, 2).reshape(128, 512)
        P[:, 512:1024] = w2t.reshape(2, 128, H).transpose(1, 0, 2).reshape(128, 512)
        P[:, 1024:1026] = wvec.reshape(2, 128).T
        P[:, 1026:1028] = bvec.reshape(2, 128).T
        P[:, 1028:1036] = (
            query[sl].T.reshape(2, 128, BL).transpose(1, 0, 2).reshape(128, 2 * BL)
        )
        return np.ascontiguousarray(P)

    in_maps = []
    for i in range(NCORES):
        sl = slice(i * BL, (i + 1) * BL)
        in_maps.append({
            "value": value[sl],
            "lens": lens[sl],
            "params": pack(i),
        })
    return in_maps


def _axon_reset():
    # clear a wedged exec unit left over from a previous crashed run
    try:
        import ctypes
        import jax
        jax.devices()
        lib = ctypes.CDLL("/opt/axon/libaxon_pjrt.so")
        lib.axon_reset.restype = ctypes.c_int64
        lib.axon_reset()
    except Exception:
        pass


def kernel(value, query, lens, W1, W2, b, w):
    nc = _get_nc()
    in_maps = make_in_maps(value, query, lens, W1, W2, b, w)
    try:
        res = run_bass_kernel_spmd(nc, in_maps, core_ids=list(range(NCORES)))
    except Exception:
        _axon_reset()
        res = run_bass_kernel_spmd(nc, in_maps, core_ids=list(range(NCORES)))
    out = np.concatenate(
        [np.asarray(res.results[i]["out"]) for i in range(NCORES)], axis=0
    )
    return out.astype(np.float32)


# revision 10
# speedup vs baseline: 1.4765x; 1.4765x over previous
"""Additive-attention pooling kernel for 8 TRN2 NeuronCores.

Problem (per full input):
    u = tanh(value @ W1^T + query @ W2^T + b)          # [B, S, H]
    scores = u @ w, masked to s < lens[b], softmax over s
    out = sum_s softmax(scores)[b, s] * value[b, s, :]  # [B, DV]

Sharding: data-parallel over the batch dim (4 batches per core); the small
parameters (W1, W2, b, w) are replicated.

Per-core pipeline (matmuls in bf16, f32 PSUM accumulation), software-
pipelined g-outer/b-inner so every phase overlaps the value-load DMA:
  1. SWDGE DMAs load value in 512KB chunks, casting f32->bf16 in the
     DMA datapath, into nat[p, t, v] = value[128t + p, v]; issue order is
     chunk-major/batch-minor so all four batches stream in lockstep.
  2. value^T tiles via DMA xbar transposes (SBUF->SBUF bf16 on the sync
     HWDGE queue): rides the 435GB/s SBUF fabric, which has headroom while
     HBM (358GB/s) is the load bottleneck, and keeps the PE free for real
     matmuls.  Output layout vt[vv, q=(t,vh), s].
  3. u-matmul per (batch, 1024-s chunk): W1T chunks stationary, vt moving;
     ScalarE tanh with per-partition bias (c = query@W2^T + b) -> uT bf16.
  4. After each chunk g completes across batches: scores matmuls (M=32,
     4 batches col-tiled concurrently), exp, xbar e-transpose (again DMA,
     straight to SBUF), DVE mask+replicate with accumulated e-sums.
  5. Pooling matmuls for chunk g's 8 s-tiles follow immediately (4 batches
     col-tiled, one PSUM bank per batch, accumulating over all 32 s-tiles).
  6. sum(e) -> reduce + one N=1 matmul per batch; reciprocal scale.
  Dummy warmup matmuls during the first loads release the PE HAM throttle
  before the real u-matmuls arrive.  All small parameters ship pre-packed
  in one [128, 1036] image so a single DMA replaces eight small ones.
"""

import numpy as np

import concourse.bass as bass
import concourse.bacc as bacc
import concourse.tile as tile
from concourse import mybir
from concourse.bass_utils import run_bass_kernel_spmd


B, S, DV, DQ, H = 32, 4096, 256, 256, 256
NCORES = 8
BL = B // NCORES  # batches per core

ST = S // 128     # 32 s-tiles of 128
NG = 4            # compute chunks per batch (1024 s each)
GT = ST // NG     # s-tiles per chunk (8)
PW = 1036         # packed params width: w1t 512 | w2t 512 | w 2 | b 2 | qT 8
F32 = mybir.dt.float32
BF16 = mybir.dt.bfloat16
I32 = mybir.dt.int32


def build_nc():
    nc = bacc.Bacc("TRN2", target_bir_lowering=False)

    value_ext = nc.declare_dram_parameter("value", [BL, S, DV], F32, isOutput=False)
    lens_ext = nc.declare_dram_parameter("lens", [BL], I32, isOutput=False)
    params_ext = nc.declare_dram_parameter(
        "params", [128, PW], F32, isOutput=False
    )
    out_ext = nc.declare_dram_parameter("out", [BL, DV], F32, isOutput=True)

    Tanh = mybir.ActivationFunctionType.Tanh
    Exp = mybir.ActivationFunctionType.Exp
    Alu = mybir.AluOpType

    with tile.TileContext(nc) as tc:
        with (
            tc.tile_pool(name="singles", bufs=1) as singles,
            tc.tile_pool(name="nat", bufs=BL) as nat_pool,
            tc.tile_pool(name="vt", bufs=8) as vt_pool,
            tc.tile_pool(name="et", bufs=2) as et_pool,
            tc.tile_pool(name="ut", bufs=2 * BL) as ut_pool,
        ):
            # ---- iotas first (cheap; keeps the load-DMA queue behind them short)
            io_col = singles.tile([128, 128], I32, tag="io_col")
            io_row = singles.tile([128, 128], I32, tag="io_row")
            nc.gpsimd.iota(io_col, [[1, 128]], channel_multiplier=0)
            nc.gpsimd.iota(io_row, [[0, 128]], channel_multiplier=1)
            identity = singles.tile([128, 128], BF16, tag="identity")
            nc.vector.tensor_tensor(identity, io_row, io_col, Alu.is_equal)

            # s-index iota for the length mask: val[p, t] = 128t + p
            iota_s = singles.tile([128, ST], F32, tag="iota_s")
            nc.gpsimd.iota(
                iota_s, [[128, ST]], channel_multiplier=1,
                allow_small_or_imprecise_dtypes=True,
            )

            # ---- value loads: SWDGE cast-DMAs (f32->bf16), chunk-major/
            # batch-minor so the four batches' chunk g data all arrives early
            nat = []
            for b in range(BL):
                natb = nat_pool.tile([128, ST, DV], BF16, tag="nat")
                nat.append(natb)
            for ch in range(8):
                for b in range(BL):
                    src = value_ext[b, ch * 512:(ch + 1) * 512, :]
                    nc.gpsimd.dma_start(
                        out=nat[b][:, ch * 4:(ch + 1) * 4, :],
                        in_=src.rearrange("(t p) v -> p t v", p=128),
                    )

            params_sb = singles.tile([128, PW], F32, tag="params_sb")
            nc.sync.dma_start(out=params_sb, in_=params_ext[:, :])
            w1t_f = params_sb[:, 0:512].rearrange("p (c h) -> p c h", c=2)
            w2t_f = params_sb[:, 512:1024].rearrange("p (c h) -> p c h", c=2)
            w_f = params_sb[:, 1024:1026]
            b_sb = params_sb[:, 1026:1028]
            qT = params_sb[:, 1028:1036].rearrange("p (c b) -> p c b", c=2)

            lens_i = singles.tile([128, BL], I32, tag="lens_i")
            nc.sync.dma_start(
                out=lens_i,
                in_=bass.AP(tensor=lens_ext, offset=0, ap=[[0, 128], [1, BL]]),
            )
            lens_f = singles.tile([128, BL], F32, tag="lens_f")
            nc.vector.tensor_copy(lens_f, lens_i)

            w1t_bf = singles.tile([128, 2, H], BF16, tag="w1t_bf")
            nc.vector.tensor_copy(w1t_bf, w1t_f)

            zero32 = singles.tile([128, 32], BF16, tag="zero32")
            nc.vector.memset(zero32, 0.0)
            w_rep = singles.tile([128, 2, 32], BF16, tag="w_rep")
            for hh in range(2):
                nc.vector.tensor_scalar(
                    w_rep[:, hh, :], zero32, w_f[:, hh:hh + 1], None, Alu.add
                )

            # 1/32-filled stationary for the sum(e) matmul
            ones_rep = singles.tile([128, 32], BF16, tag="ones_rep")
            nc.vector.memset(ones_rep, 1.0 / 32.0)

            # c[b, h] = query[b] @ W2^T + b   ->  cT [128h, hh, b] f32
            # plus PE warmup: junk matmuls so the HAM clock-gate releases
            # (1.2 -> 2.4 GHz) while the first value chunks are still loading
            cT = singles.tile([128, 2, BL], F32, tag="cT")
            with tc.tile_pool(name="ct_ps", bufs=2, space="PSUM") as ct_pool:
                warm_ps = ct_pool.tile([128, 128], F32, tag="warm")
                for i in range(36):
                    nc.tensor.matmul(
                        warm_ps,
                        w2t_f[:, 0, 0:128],
                        w1t_f[:, 0, 0:128],
                        start=True,
                        stop=True,
                    )
                for hh in range(2):
                    ct_ps = ct_pool.tile([128, BL], F32, tag="ct")
                    for c in range(2):
                        nc.tensor.matmul(
                            ct_ps,
                            w2t_f[:, c, hh * 128:(hh + 1) * 128],
                            qT[:, c, :],
                            start=(c == 0),
                            stop=(c == 1),
                        )
                    nc.vector.tensor_scalar(
                        cT[:, hh, :], ct_ps, b_sb[:, hh:hh + 1], None, Alu.add
                    )

            # ---- main pipelined loop -----------------------------------
            ut = [None] * (2 * BL)
            e_sb = singles.tile([128, S], BF16, tag="e_sb")
            e_resh = singles.tile([128, ST, BL], BF16, tag="e_resh")
            e_rep = singles.tile([128, BL, ST, 32], BF16, tag="e_rep")
            psums = singles.tile([128, BL, 8], F32, tag="psums")
            psums_r = singles.tile([128, BL], F32, tag="psums_r")
            psums_bf = singles.tile([128, BL], BF16, tag="psums_bf")
            out_sb = singles.tile([128, DV], F32, tag="out_sb")
            sums_r = singles.tile([128, 1], F32, tag="sums_r")

            with (
                tc.tile_pool(name="up_ps", bufs=2, space="PSUM") as up_pool,
                tc.tile_pool(name="wk_ps", bufs=2, space="PSUM") as wk_pool,
                tc.tile_pool(name="po_ps", bufs=1, space="PSUM") as po_pool,
            ):
                # pooling accumulator: batch b col-tiled into partitions
                # [32b, 32b+32), bank b, cols [0, DV] (+ col DV = sum e)
                po_ps = po_pool.tile([128, BL, 512], F32, tag="po")

                def emit_e_chain(e8):
                    soff = e8 * 512
                    toff = e8 * 4
                    sc_ps = wk_pool.tile([128, 512], F32, tag="wk", name=f"sc{e8}")
                    for bb in range(BL):
                        for hh in range(2):
                            nc.tensor.matmul(
                                sc_ps[32 * bb:32 * bb + 32, :],
                                w_rep[:, hh, :],
                                ut[2 * bb + hh][:, soff:soff + 512],
                                start=(hh == 0),
                                stop=(hh == 1),
                                tile_position=(0, 32 * bb),
                            )
                    nc.scalar.activation(e_sb[:, soff:soff + 512], sc_ps, Exp)
                    # e-transpose via DMA xbar: et[x, q, k] = e_sb[k, soff+128q+x]
                    # (k = 32*bb + rep; column 32*bb holds batch bb's e values)
                    et = et_pool.tile([128, 4, 128], BF16, tag="et")
                    nc.sync.dma_start(
                        out=et,
                        in_=e_sb[:, soff:soff + 512],
                        transpose=True,
                    )
                    nc.vector.tensor_copy(
                        e_resh[:, toff:toff + 4, :],
                        et.rearrange("p q (bb x) -> p q bb x", x=32)[:, :, :, 0],
                    )
                    for bb in range(BL):
                        er = e_resh[:, toff:toff + 4, bb]
                        er_b = bass.AP(tensor=er.tensor, offset=er.offset,
                                       ap=[*er.ap, [0, 32]])
                        io = iota_s[:, toff:toff + 4]
                        io_b = bass.AP(tensor=io.tensor, offset=io.offset,
                                       ap=[*io.ap, [0, 32]])
                        nc.vector.scalar_tensor_tensor(
                            e_rep[:, bb, toff:toff + 4],
                            io_b,
                            lens_f[:, bb:bb + 1],
                            er_b,
                            Alu.is_lt,
                            Alu.mult,
                            accum_out=psums[:, bb, e8:e8 + 1],
                        )

                for g in range(NG):
                    t0 = g * GT
                    for b in range(BL):
                        # --- value^T tiles for (b, g): [vv, q=(t',vh), s]
                        # PE identity-transposes (bf16 PSUM), DVE evacuates
                        vt = vt_pool.tile([128, 2 * GT, 128], BF16, tag="vt")
                        vt_q = vt.rearrange("p (t two) c -> p two t c", two=2)
                        for vh in range(2):
                            tp = wk_pool.tile(
                                [128, 1024], BF16, tag="wk", name=f"tp{g}_{b}_{vh}"
                            )
                            for k in range(GT):
                                nc.tensor.matmul(
                                    tp[:, k * 128:(k + 1) * 128],
                                    nat[b][:, t0 + k, vh * 128:(vh + 1) * 128],
                                    identity,
                                    is_transpose=True,
                                    start=(k % 4 == 0),
                                    stop=(k % 4 == 3),
                                )
                            tp_t = tp.rearrange("p (t c) -> p t c", c=128)
                            nc.vector.tensor_copy(vt_q[:, vh], tp_t)
                        # --- u-matmul + tanh for (b, g) ------------------
                        for hh in range(2):
                            if g == 0:
                                utb = ut_pool.tile([128, S], BF16, tag="ut")
                                ut[2 * b + hh] = utb
                            utb = ut[2 * b + hh]
                            for sc in range(2):
                                up = up_pool.tile([128, 512], F32, tag="up")
                                for vh in range(2):
                                    mv = vt_q[:, vh, sc * 4:(sc + 1) * 4, :]
                                    nc.tensor.matmul(
                                        up,
                                        w1t_bf[:, vh, hh * 128:(hh + 1) * 128],
                                        mv,
                                        start=(vh == 0),
                                        stop=(vh == 1),
                                    )
                                so = g * 1024 + sc * 512
                                nc.scalar.activation(
                                    utb[:, so:so + 512],
                                    up,
                                    Tanh,
                                    bias=cT[:, hh, b:b + 1],
                                    scale=1.0,
                                )

                    # --- scores / e for chunk g across all batches -------
                    emit_e_chain(2 * g)
                    emit_e_chain(2 * g + 1)

                    # --- pooling for chunk g's s-tiles -------------------
                    for k in range(GT):
                        t = t0 + k
                        for b in range(BL):
                            nc.tensor.matmul(
                                po_ps[32 * b:32 * b + 32, b, 0:DV],
                                e_rep[:, b, t, :],
                                nat[b][:, t, :],
                                start=(t == 0),
                                stop=(t == ST - 1),
                                tile_position=(0, 32 * b),
                            )

                # ---- finalize: sum(e), normalize, store ----------------
                nc.vector.tensor_reduce(
                    psums_r, psums, op=Alu.add, axis=mybir.AxisListType.X
                )
                nc.vector.tensor_copy(psums_bf, psums_r)
                for b in range(BL):
                    nc.tensor.matmul(
                        po_ps[32 * b:32 * b + 32, b, DV:DV + 1],
                        ones_rep,
                        psums_bf[:, b:b + 1],
                        start=True,
                        stop=True,
                        tile_position=(0, 32 * b),
                    )
                for b in range(BL):
                    rows = slice(32 * b, 32 * b + 32)
                    nc.vector.reciprocal(
                        sums_r[rows], po_ps[rows, b, DV:DV + 1]
                    )
                    nc.vector.tensor_scalar(
                        out_sb[rows], po_ps[rows, b, 0:DV], sums_r[rows],
                        None, Alu.mult,
                    )
                ob_rows = out_sb.rearrange("(a b) s -> a b s", b=32)[:, 0, :]
                nc.sync.dma_start(out=out_ext[:, :], in_=ob_rows)

    nc.compile()
    return nc


_NC_CACHE = None


def _get_nc():
    global _NC_CACHE
    if _NC_CACHE is None:
        _NC_CACHE = build_nc()
    return _NC_CACHE


def make_in_maps(value, query, lens, W1, W2, b, w):
    value = np.ascontiguousarray(np.asarray(value, dtype=np.float32))
    query = np.asarray(query, dtype=np.float32)
    lens = np.ascontiguousarray(np.asarray(lens, dtype=np.int32))
    w1t = np.asarray(W1, dtype=np.float32).T
    w2t = np.asarray(W2, dtype=np.float32).T
    bvec = np.asarray(b, dtype=np.float32).reshape(H)
    wvec = np.asarray(w, dtype=np.float32).reshape(H)

    def pack(core):
        sl = slice(core * BL, (core + 1) * BL)
        P = np.zeros((128, PW), np.float32)
        P[:, 0:512] = w1t.reshape(2, 128, H).transpose(1, 0, 2).reshape(128, 512)
        P[:, 512:1024] = w2t.reshape(2, 128, H).transpose(1, 0, 2).reshape(128, 512)
        P[:, 1024:1026] = wvec.reshape(2, 128).T
        P[:, 1026:1028] = bvec.reshape(2, 128).T
        P[:, 1028:1036] = (
            query[sl].T.reshape(2, 128, BL).transpose(1, 0, 2).reshape(128, 2 * BL)
        )
        return np.ascontiguousarray(P)

    in_maps = []
    for i in range(NCORES):
        sl = slice(i * BL, (i + 1) * BL)
        in_maps.append({
            "value": value[sl],
            "lens": lens[sl],
            "params": pack(i),
        })
    return in_maps


def _axon_reset():
    # clear a wedged exec unit left over from a previous crashed run
    try:
        import ctypes
        import jax
        jax.devices()
        lib = ctypes.CDLL("/opt/axon/libaxon_pjrt.so")
        lib.axon_reset.restype = ctypes.c_int64
        lib.axon_reset()
    except Exception:
        pass


def kernel(value, query, lens, W1, W2, b, w):
    nc = _get_nc()
    in_maps = make_in_maps(value, query, lens, W1, W2, b, w)
    try:
        res = run_bass_kernel_spmd(nc, in_maps, core_ids=list(range(NCORES)))
    except Exception:
        _axon_reset()
        res = run_bass_kernel_spmd(nc, in_maps, core_ids=list(range(NCORES)))
    out = np.concatenate(
        [np.asarray(res.results[i]["out"]) for i in range(NCORES)], axis=0
    )
    return out.astype(np.float32)


# revision 11
# speedup vs baseline: 2.3920x; 1.6200x over previous
"""Additive-attention pooling kernel for 8 TRN2 NeuronCores.

Problem (per full input):
    u = tanh(value @ W1^T + query @ W2^T + b)          # [B, S, H]
    scores = u @ w, masked to s < lens[b], softmax over s
    out = sum_s softmax(scores)[b, s] * value[b, s, :]  # [B, DV]

Sharding: data-parallel over the batch dim (4 batches per core); the small
parameters (W1, W2, b, w) are replicated.

Per-core pipeline (matmuls in bf16, f32 PSUM accumulation), software-
pipelined g-outer/b-inner so the score/e chains overlap the value-load DMA
instead of piling up in a serial tail:
  1. SWDGE DMAs load value in 512KB chunks, casting f32->bf16 in the DMA
     datapath, into nat[p, t, v] = value[128t + p, v]; issue order matches
     the compute order (chunk-pair major, batch minor).
  2. TensorE identity-transposes (transpose mode, bf16 PSUM, 4-tile
     accumulation groups) produce valueT tiles; VectorE evacuates.
  3. u-matmul per (batch, 1024-s chunk): W1T chunks stationary, valueT
     moving; ScalarE tanh with per-partition bias (c = query@W2^T + b)
     writes uT bf16 to SBUF.
  4. After chunk g completes across all batches (one g late, so the PE
     queue never head-blocks): scores matmuls (M=32, 4 batches col-tiled
     concurrently), exp, PE e-transpose, DVE mask+replicate with
     accumulated per-partition e-sums.
  5. Tail: pooling matmuls (M=32, 4 batches col-tiled, one PSUM bank per
     batch) accumulate over all 32 s-tiles; sum(e) via reduce + one N=1
     matmul per batch; reciprocal scale finishes the softmax.
  Dummy bf16 warmup matmuls during the first loads release the PE HAM
  clock throttle (1.2 -> 2.4 GHz) before the real transposes arrive.
  All small parameters ship pre-packed in one [128, 1036] image so a
  single DMA replaces eight small ones on the Sync queue.
"""

import numpy as np

import concourse.bass as bass
import concourse.bacc as bacc
import concourse.tile as tile
from concourse import mybir
from concourse.bass_utils import run_bass_kernel_spmd


B, S, DV, DQ, H = 32, 4096, 256, 256, 256
NCORES = 8
BL = B // NCORES  # batches per core

ST = S // 128     # 32 s-tiles of 128
NG = 4            # compute chunks per batch (1024 s each)
GT = ST // NG     # s-tiles per chunk (8)
PW = 1036         # packed params width: w1t 512 | w2t 512 | w 2 | b 2 | qT 8
F32 = mybir.dt.float32
BF16 = mybir.dt.bfloat16
I32 = mybir.dt.int32


def build_nc():
    nc = bacc.Bacc("TRN2", target_bir_lowering=False)

    value_ext = nc.declare_dram_parameter("value", [BL, S, DV], F32, isOutput=False)
    lens_ext = nc.declare_dram_parameter("lens", [BL], I32, isOutput=False)
    params_ext = nc.declare_dram_parameter(
        "params", [128, PW], F32, isOutput=False
    )
    out_ext = nc.declare_dram_parameter("out", [BL, DV], F32, isOutput=True)

    Tanh = mybir.ActivationFunctionType.Tanh
    Exp = mybir.ActivationFunctionType.Exp
    Alu = mybir.AluOpType

    with tile.TileContext(nc) as tc:
        with (
            tc.tile_pool(name="singles", bufs=1) as singles,
            tc.tile_pool(name="nat", bufs=BL) as nat_pool,
            tc.tile_pool(name="vt", bufs=8) as vt_pool,
            tc.tile_pool(name="ut", bufs=2 * BL) as ut_pool,
        ):
            # ---- iotas first (cheap; keeps the load-DMA queue behind them short)
            io_col = singles.tile([128, 128], I32, tag="io_col")
            io_row = singles.tile([128, 128], I32, tag="io_row")
            nc.gpsimd.iota(io_col, [[1, 128]], channel_multiplier=0)
            nc.gpsimd.iota(io_row, [[0, 128]], channel_multiplier=1)
            identity = singles.tile([128, 128], BF16, tag="identity")
            nc.vector.tensor_tensor(identity, io_row, io_col, Alu.is_equal)

            # s-index iota for the length mask: val[p, t] = 128t + p
            iota_s = singles.tile([128, ST], F32, tag="iota_s")
            nc.gpsimd.iota(
                iota_s, [[128, ST]], channel_multiplier=1,
                allow_small_or_imprecise_dtypes=True,
            )

            # ---- value loads: SWDGE cast-DMAs (f32->bf16), issued in the
            # order compute consumes them (chunk-pair g major, batch minor)
            nat = []
            for b in range(BL):
                natb = nat_pool.tile([128, ST, DV], BF16, tag="nat")
                nat.append(natb)
            for g in range(NG):
                for b in range(BL):
                    for ch in (2 * g, 2 * g + 1):
                        src = value_ext[b, ch * 512:(ch + 1) * 512, :]
                        nc.gpsimd.dma_start(
                            out=nat[b][:, ch * 4:(ch + 1) * 4, :],
                            in_=src.rearrange("(t p) v -> p t v", p=128),
                        )

            params_sb = singles.tile([128, PW], F32, tag="params_sb")
            nc.sync.dma_start(out=params_sb, in_=params_ext[:, :])
            w1t_f = params_sb[:, 0:512].rearrange("p (c h) -> p c h", c=2)
            w2t_f = params_sb[:, 512:1024].rearrange("p (c h) -> p c h", c=2)
            w_f = params_sb[:, 1024:1026]
            b_sb = params_sb[:, 1026:1028]
            qT = params_sb[:, 1028:1036].rearrange("p (c b) -> p c b", c=2)

            lens_i = singles.tile([128, BL], I32, tag="lens_i")
            nc.sync.dma_start(
                out=lens_i,
                in_=bass.AP(tensor=lens_ext, offset=0, ap=[[0, 128], [1, BL]]),
            )
            lens_f = singles.tile([128, BL], F32, tag="lens_f")
            nc.vector.tensor_copy(lens_f, lens_i)

            w1t_bf = singles.tile([128, 2, H], BF16, tag="w1t_bf")
            nc.vector.tensor_copy(w1t_bf, w1t_f)

            zero32 = singles.tile([128, 32], BF16, tag="zero32")
            nc.vector.memset(zero32, 0.0)
            w_rep = singles.tile([128, 2, 32], BF16, tag="w_rep")
            for hh in range(2):
                nc.vector.tensor_scalar(
                    w_rep[:, hh, :], zero32, w_f[:, hh:hh + 1], None, Alu.add
                )

            # 1/32-filled stationary for the sum(e) matmul
            ones_rep = singles.tile([128, 32], BF16, tag="ones_rep")
            nc.vector.memset(ones_rep, 1.0 / 32.0)

            # c[b, h] = query[b] @ W2^T + b   ->  cT [128h, hh, b] f32
            # plus bf16 PE warmup so the HAM clock-gate releases during loads
            cT = singles.tile([128, 2, BL], F32, tag="cT")
            with tc.tile_pool(name="ct_ps", bufs=2, space="PSUM") as ct_pool:
                warm_ps = ct_pool.tile([128, 128], F32, tag="warm")
                for i in range(40):
                    nc.tensor.matmul(
                        warm_ps,
                        w1t_bf[:, 0, 0:128],
                        identity,
                        start=True,
                        stop=True,
                    )
                for hh in range(2):
                    ct_ps = ct_pool.tile([128, BL], F32, tag="ct")
                    for c in range(2):
                        nc.tensor.matmul(
                            ct_ps,
                            w2t_f[:, c, hh * 128:(hh + 1) * 128],
                            qT[:, c, :],
                            start=(c == 0),
                            stop=(c == 1),
                        )
                    nc.vector.tensor_scalar(
                        cT[:, hh, :], ct_ps, b_sb[:, hh:hh + 1], None, Alu.add
                    )

            # ---- phase A: transpose + u-matmul + tanh, g-outer/b-inner,
            # with e-chains interleaved one chunk late ---------------------
            ut = [None] * (2 * BL)
            e_sb = singles.tile([128, S], BF16, tag="e_sb")
            e_resh = singles.tile([128, ST, BL], BF16, tag="e_resh")
            e_rep = singles.tile([128, BL, ST, 32], BF16, tag="e_rep")
            psums = singles.tile([128, BL, 8], F32, tag="psums")
            with (
                tc.tile_pool(name="tp_ps", bufs=2, space="PSUM") as tp_pool,
                tc.tile_pool(name="up_ps", bufs=2, space="PSUM") as up_pool,
                tc.tile_pool(name="se_ps", bufs=2, space="PSUM") as se_pool,
            ):
                def emit_e_chain(e8):
                    soff = e8 * 512
                    toff = e8 * 4
                    sc_ps = se_pool.tile([128, 512], F32, tag="se", name=f"sc{e8}")
                    for bb in range(BL):
                        for hh in range(2):
                            nc.tensor.matmul(
                                sc_ps[32 * bb:32 * bb + 32, :],
                                w_rep[:, hh, :],
                                ut[2 * bb + hh][:, soff:soff + 512],
                                start=(hh == 0),
                                stop=(hh == 1),
                                tile_position=(0, 32 * bb),
                            )
                    nc.scalar.activation(e_sb[:, soff:soff + 512], sc_ps, Exp)
                    et = se_pool.tile([128, 512], F32, tag="se", name=f"et{e8}")
                    etb = et.bitcast(BF16)[:, 0:512]
                    for j in range(4):
                        nc.tensor.matmul(
                            etb[:, j * 128:(j + 1) * 128],
                            e_sb[:, soff + j * 128:soff + (j + 1) * 128],
                            identity,
                            is_transpose=True,
                            start=(j == 0),
                            stop=(j == 3),
                        )
                    ev = etb.rearrange("p (t c) -> p t c", c=128)
                    nc.vector.tensor_copy(
                        e_resh[:, toff:toff + 4, :],
                        ev.rearrange("p t (bb x) -> p t bb x", x=32)[:, :, :, 0],
                    )
                    for bb in range(BL):
                        er = e_resh[:, toff:toff + 4, bb]
                        er_b = bass.AP(tensor=er.tensor, offset=er.offset,
                                       ap=[*er.ap, [0, 32]])
                        io = iota_s[:, toff:toff + 4]
                        io_b = bass.AP(tensor=io.tensor, offset=io.offset,
                                       ap=[*io.ap, [0, 32]])
                        nc.vector.scalar_tensor_tensor(
                            e_rep[:, bb, toff:toff + 4],
                            io_b,
                            lens_f[:, bb:bb + 1],
                            er_b,
                            Alu.is_lt,
                            Alu.mult,
                            accum_out=psums[:, bb, e8:e8 + 1],
                        )

                for g in range(NG):
                    t0 = g * GT
                    for b in range(BL):
                        vts = {}
                        for vh in range(2):
                            vt = vt_pool.tile([128, 1024], BF16, tag="vt")
                            vts[vh] = vt
                            tp = tp_pool.tile([128, 1024], BF16, tag="tp")
                            for k in range(GT):
                                nc.tensor.matmul(
                                    tp[:, k * 128:(k + 1) * 128],
                                    nat[b][:, t0 + k, vh * 128:(vh + 1) * 128],
                                    identity,
                                    is_transpose=True,
                                    start=(k % 4 == 0),
                                    stop=(k % 4 == 3),
                                )
                            nc.vector.tensor_copy(vt, tp)
                        for hh in range(2):
                            if g == 0:
                                utb = ut_pool.tile([128, S], BF16, tag="ut")
                                ut[2 * b + hh] = utb
                            utb = ut[2 * b + hh]
                            up = up_pool.tile([128, 1024], F32, tag="up")
                            for sc in range(2):
                                lo = sc * 512
                                for vh in range(2):
                                    nc.tensor.matmul(
                                        up[:, lo:lo + 512],
                                        w1t_bf[:, vh, hh * 128:(hh + 1) * 128],
                                        vts[vh][:, lo:lo + 512],
                                        start=(vh == 0),
                                        stop=(vh == 1),
                                    )
                            nc.scalar.activation(
                                utb[:, g * 1024:(g + 1) * 1024],
                                up,
                                Tanh,
                                bias=cT[:, hh, b:b + 1],
                                scale=1.0,
                            )
                    if g > 0:
                        emit_e_chain(2 * (g - 1))
                        emit_e_chain(2 * (g - 1) + 1)
                emit_e_chain(2 * (NG - 1))
                emit_e_chain(2 * (NG - 1) + 1)

            # ---- phase C: pooling + normalization ----------------------
            psums_r = singles.tile([128, BL], F32, tag="psums_r")
            psums_bf = singles.tile([128, BL], BF16, tag="psums_bf")
            out_sb = singles.tile([128, DV], F32, tag="out_sb")
            sums_r = singles.tile([128, 1], F32, tag="sums_r")

            with tc.tile_pool(name="po_ps", bufs=1, space="PSUM") as po_pool:
                po_ps = po_pool.tile([128, BL, 512], F32, tag="po")
                for t in range(ST):
                    for b in range(BL):
                        nc.tensor.matmul(
                            po_ps[32 * b:32 * b + 32, b, 0:DV],
                            e_rep[:, b, t, :],
                            nat[b][:, t, :],
                            start=(t == 0),
                            stop=(t == ST - 1),
                            tile_position=(0, 32 * b),
                        )

                # sum(e): per-partition sums -> reduce over eighths -> bf16
                # -> one N=1 matmul per batch into po column DV
                nc.vector.tensor_reduce(
                    psums_r, psums, op=Alu.add, axis=mybir.AxisListType.X
                )
                nc.vector.tensor_copy(psums_bf, psums_r)
                for b in range(BL):
                    nc.tensor.matmul(
                        po_ps[32 * b:32 * b + 32, b, DV:DV + 1],
                        ones_rep,
                        psums_bf[:, b:b + 1],
                        start=True,
                        stop=True,
                        tile_position=(0, 32 * b),
                    )
                for b in range(BL):
                    rows = slice(32 * b, 32 * b + 32)
                    nc.vector.reciprocal(
                        sums_r[rows], po_ps[rows, b, DV:DV + 1]
                    )
                    nc.vector.tensor_scalar(
                        out_sb[rows], po_ps[rows, b, 0:DV], sums_r[rows],
                        None, Alu.mult,
                    )
                ob_rows = out_sb.rearrange("(a b) s -> a b s", b=32)[:, 0, :]
                nc.sync.dma_start(out=out_ext[:, :], in_=ob_rows)

    nc.compile()
    return nc


_NC_CACHE = None


def _get_nc():
    global _NC_CACHE
    if _NC_CACHE is None:
        _NC_CACHE = build_nc()
    return _NC_CACHE


def make_in_maps(value, query, lens, W1, W2, b, w):
    value = np.ascontiguousarray(np.asarray(value, dtype=np.float32))
    query = np.asarray(query, dtype=np.float32)
    lens = np.ascontiguousarray(np.asarray(lens, dtype=np.int32))
    w1t = np.asarray(W1, dtype=np.float32).T
    w2t = np.asarray(W2, dtype=np.float32).T
    bvec = np.asarray(b, dtype=np.float32).reshape(H)
    wvec = np.asarray(w, dtype=np.float32).reshape(H)

    def pack(core):
        sl = slice(core * BL, (core + 1) * BL)
        P = np.zeros((128, PW), np.float32)
        P[:, 0:512] = w1t.reshape(2, 128, H).transpose(1, 0, 2).reshape(128, 512)
        P[:, 512:1024] = w2t.reshape(2, 128, H).transpose(1, 0, 2).reshape(128, 512)
        P[:, 1024:1026] = wvec.reshape(2, 128).T
        P[:, 1026:1028] = bvec.reshape(2, 128).T
        P[:, 1028:1036] = (
            query[sl].T.reshape(2, 128, BL).transpose(1, 0, 2).reshape(128, 2 * BL)
        )
        return np.ascontiguousarray(P)

    in_maps = []
    for i in range(NCORES):
        sl = slice(i * BL, (i + 1) * BL)
        in_maps.append({
            "value": value[sl],
            "lens": lens[sl],
            "params": pack(i),
        })
    return in_maps


def _axon_reset():
    # clear a wedged exec unit left over from a previous crashed run
    try:
        import ctypes
        import jax
        jax.devices()
        lib = ctypes.CDLL("/opt/axon/libaxon_pjrt.so")
        lib.axon_reset.restype = ctypes.c_int64
        lib.axon_reset()
    except Exception:
        pass


def kernel(value, query, lens, W1, W2, b, w):
    nc = _get_nc()
    in_maps = make_in_maps(value, query, lens, W1, W2, b, w)
    try:
        res = run_bass_kernel_spmd(nc, in_maps, core_ids=list(range(NCORES)))
    except Exception:
        _axon_reset()
        res = run_bass_kernel_spmd(nc, in_maps, core_ids=list(range(NCORES)))
    out = np.concatenate(
        [np.asarray(res.results[i]["out"]) for i in range(NCORES)], axis=0
    )
    return out.astype(np.float32)
